# revision 1
# baseline (speedup 1.0000x reference)
"""MLA (multi-head latent attention) prefill block on 8 Trainium2 NeuronCores.

Tensor-parallel over heads: each core computes 4 of the 32 heads end-to-end
(q projection, absorbed q, latent attention, head output projection, and its
partial slice of the output projection). The kv latent path (kv_a projection,
rms-norm, rope) is replicated on every core. Per-core partial outputs (the
row-parallel wo matmul) are summed on the host.

Everything on-device is computed transposed ([feature, seq] layouts) so that
no activation transposes are needed except kv_c / k_pe (done once via the PE
transpose path, shared by all heads).

Self-contained: hardcodes all shapes from the problem spec.
"""

import os
from contextlib import ExitStack

import numpy as np

import concourse.bacc as bacc
import concourse.bass as bass
import concourse.mybir as mybir
import concourse.tile as tile
from concourse.bass_utils import run_bass_kernel_spmd
from concourse.masks import make_identity

# ---- problem constants ----
DIM = 2048
NH = 32
DN = 128  # qk_nope_head_dim
DR = 64   # qk_rope_head_dim
DV = 128  # v_head_dim
KVL = 512  # kv_lora_rank
S = 2048   # sequence length (B=1)
SCALE = float((DN + DR) ** -0.5)
EPS = 1e-6

NCORES = 8
NHC = NH // NCORES      # heads per core = 4
P = 128                 # partitions
SF = 512                # free-dim tile (s tiles)
NST = S // SF           # 4 s tiles
NTT = S // P            # 16 t tiles
NDC = DIM // P          # 16 contraction chunks over model dim
NCC = KVL // P          # 4 latent chunks

F32 = mybir.dt.float32
F32R = mybir.dt.float32r

USE_F32R = os.environ.get("MLA_F32R", "1") == "1"
RT = F32R if USE_F32R else F32  # dtype for all matmul operands


def build_nc(repeat=1):
    """Build the per-core Bass program (identical on all 8 cores)."""
    nc = bacc.Bacc("TRN2", target_bir_lowering=False, debug=False,
                   num_devices=NCORES)

    # ---- DRAM I/O ----
    d_xT = nc.dram_tensor("xT", [DIM, S], RT, kind="ExternalInput")
    d_wqn = nc.dram_tensor("wq_n", [DIM, NHC * DN], RT, kind="ExternalInput")
    d_wqpr = nc.dram_tensor("wq_pr", [DIM, NHC * 32], RT, kind="ExternalInput")
    d_wqpi = nc.dram_tensor("wq_pi", [DIM, NHC * 32], RT, kind="ExternalInput")
    d_wkva = nc.dram_tensor("wkv_a", [DIM, KVL + DR], RT, kind="ExternalInput")
    d_wbk = nc.dram_tensor("wbk", [NHC, DN, KVL], RT, kind="ExternalInput")
    d_wbvT = nc.dram_tensor("wbvT", [NHC, KVL, DV], RT, kind="ExternalInput")
    d_wo = nc.dram_tensor("wo_c", [NHC * DV, DIM], RT, kind="ExternalInput")
    d_cosn = nc.dram_tensor("cos_n", [S, DR // 2], F32, kind="ExternalInput")
    d_sinn = nc.dram_tensor("sin_n", [S, DR // 2], F32, kind="ExternalInput")
    d_cosr = nc.dram_tensor("cosR", [P, S], F32, kind="ExternalInput")
    d_sinr = nc.dram_tensor("sinR", [P, S], F32, kind="ExternalInput")
    d_out = nc.dram_tensor("outT", [DIM, S], F32, kind="ExternalOutput")
    # scratch for q while xT occupies SBUF
    d_qns = nc.dram_tensor("qn_scratch", [NHC, DN, S], RT)
    d_qps = nc.dram_tensor("qp_scratch", [NHC, DR, S], RT)

    xT = d_xT.ap()
    out = d_out.ap()

    with tile.TileContext(nc) as tc:
      for _rep in range(repeat):
        with ExitStack() as top:
            cst = top.enter_context(tc.tile_pool(name="const", bufs=1))
            ident = cst.tile([P, P], F32, tag="ident", name="ident")
            make_identity(nc, ident[:])
            ones_f = cst.tile([P, 1], F32, tag="ones_f", name="ones_f")
            nc.gpsimd.memset(ones_f[:], 1.0)
            ones_c = cst.tile([P, 1], RT, tag="ones_c", name="ones_c")
            nc.scalar.copy(ones_c[:], ones_f[:])
            ones_r = cst.tile([1, P], F32, tag="ones_r", name="ones_r")
            nc.gpsimd.memset(ones_r[:], 1.0)
            epsb = cst.tile([P, 1], F32, tag="epsb", name="epsb")
            nc.gpsimd.memset(epsb[:], EPS)

            # long-lived: normalized latent kv (natural layout)
            kvp = top.enter_context(tc.tile_pool(name="kv", bufs=NTT))
            kvc = [kvp.tile([P, KVL], RT, tag="kvc", name="kvc")
                   for _ in range(NTT)]
            kpp = top.enter_context(tc.tile_pool(name="kpe", bufs=NTT))
            kpe = [kpp.tile([P, DR], F32, tag="kpe", name="kpe")
                   for _ in range(NTT)]

            # ===== phase 1: q + kv projections, single pass over xT ========
            with ExitStack() as ph1:
                wrp = ph1.enter_context(tc.tile_pool(name="wres", bufs=1))
                xsl0 = ph1.enter_context(tc.tile_pool(name="xsl", bufs=6))
                xTj0 = d_xT.ap()[:, 0:SF].rearrange("(d p) f -> p d f", p=P)
                xh0 = [xsl0.tile([P, 4 * SF], RT, tag="xsl", name="xsl")
                       for _ in range(4)]
                wqn_a = wrp.tile([P, NDC * NHC * DN], RT, tag="wqn",
                                 name="wqn")
                wqpr_a = wrp.tile([P, NDC * NHC * 32], RT, tag="wqpr",
                                  name="wqpr")
                wqpi_a = wrp.tile([P, NDC * NHC * 32], RT, tag="wqpi",
                                  name="wqpi")
                wkva_a = wrp.tile([P, NDC * (KVL + DR)], RT, tag="wkva",
                                  name="wkva")
                # interleave x and weight quarters in consumption order so
                # the PE starts after ~2 quarters instead of the full set
                for q4 in range(4):
                    hd = slice(q4 * (NDC // 4), (q4 + 1) * (NDC // 4))
                    nc.sync.dma_start(
                        xh0[q4][:].rearrange("p (d f) -> p d f", d=4),
                        xTj0[:, 4 * q4:4 * (q4 + 1)])
                    nc.sync.dma_start(
                        wqn_a[:].rearrange("p (d c) -> p d c", d=NDC)[:, hd],
                        d_wqn.ap().rearrange("(d p) c -> p d c", p=P)[:, hd])
                    nc.sync.dma_start(
                        wqpr_a[:].rearrange("p (d c) -> p d c", d=NDC)[:, hd],
                        d_wqpr.ap().rearrange("(d p) c -> p d c", p=P)[:, hd])
                    nc.sync.dma_start(
                        wqpi_a[:].rearrange("p (d c) -> p d c", d=NDC)[:, hd],
                        d_wqpi.ap().rearrange("(d p) c -> p d c", p=P)[:, hd])
                    nc.sync.dma_start(
                        wkva_a[:].rearrange("p (d c) -> p d c", d=NDC)[:, hd],
                        d_wkva.ap().rearrange("(d p) c -> p d c", p=P)[:, hd])
                cna = wrp.tile([P, NTT * 32], F32, tag="cna", name="cna")
                sna = wrp.tile([P, NTT * 32], F32, tag="sna", name="sna")
                nc.sync.dma_start(
                    cna[:].rearrange("p (t k) -> p t k", t=NTT),
                    d_cosn.ap().rearrange("(t p) k -> p t k", p=P))
                nc.sync.dma_start(
                    sna[:].rearrange("p (t k) -> p t k", t=NTT),
                    d_sinn.ap().rearrange("(t p) k -> p t k", p=P))

                xsl = xsl0
                stg = ph1.enter_context(tc.tile_pool(name="stg", bufs=1))
                rts = ph1.enter_context(tc.tile_pool(name="ropetmp", bufs=1))
                rox = ph1.enter_context(tc.tile_pool(name="ropeout", bufs=1))
                sqs = ph1.enter_context(tc.tile_pool(name="sqs", bufs=2))
                crs = ph1.enter_context(tc.tile_pool(name="crs", bufs=2))
                kct = ph1.enter_context(tc.tile_pool(name="kct", bufs=2))
                nrm = ph1.enter_context(tc.tile_pool(name="nrm", bufs=4))

                with tc.tile_pool(name="acc1", bufs=8, space="PSUM") as qac:
                    for j in range(NST):
                        js = slice(j * SF, (j + 1) * SF)
                        xTj = d_xT.ap()[:, js].rearrange(
                            "(d p) f -> p d f", p=P)
                        if j == 0:
                            xh = xh0
                        else:
                            xh = [xsl.tile([P, 4 * SF], RT, tag="xsl",
                                           name="xsl") for _ in range(4)]
                            for q4 in range(4):
                                nc.sync.dma_start(
                                    xh[q4][:].rearrange(
                                        "p (d f) -> p d f", d=4),
                                    xTj[:, 4 * q4:4 * (q4 + 1)])
                        # ---- q projections for this s block ----
                        pss = [qac.tile([P, SF], F32, tag="acc", name="acc")
                               for _ in range(NHC + 2)]
                        for d in range(NDC):
                            xs = xh[d // 4][:, (d % 4) * SF:(d % 4 + 1) * SF]
                            for h in range(NHC):
                                nc.tensor.matmul(
                                    pss[h][:],
                                    wqn_a[:, d * NHC * DN + h * DN:
                                          d * NHC * DN + (h + 1) * DN],
                                    xs,
                                    start=(d == 0), stop=(d == NDC - 1))
                            nc.tensor.matmul(
                                pss[NHC][:],
                                wqpr_a[:, d * P:(d + 1) * P], xs,
                                start=(d == 0), stop=(d == NDC - 1))
                            nc.tensor.matmul(
                                pss[NHC + 1][:],
                                wqpi_a[:, d * P:(d + 1) * P], xs,
                                start=(d == 0), stop=(d == NDC - 1))
                        stb = stg.tile([P, NHC * SF], RT, tag="stg",
                                       name="stg")
                        for h in range(NHC):
                            nc.scalar.copy(
                                stb[:, h * SF:(h + 1) * SF], pss[h][:])
                        nc.sync.dma_start(
                            d_qns.ap()[:, :, js].rearrange(
                                "h p f -> p h f"),
                            stb[:].rearrange("p (h f) -> p h f", h=NHC))
                        # rope rotation for q_pe (even=r, odd=i) off PSUM
                        t1 = rts.tile([P, SF], F32, tag="t1", name="t1")
                        t2 = rts.tile([P, SF], F32, tag="t2", name="t2")
                        ror = rox.tile([P, SF], RT, tag="ror", name="ror")
                        roi = rox.tile([P, SF], RT, tag="roi", name="roi")
                        cR = crs.tile([P, SF], F32, tag="cR", name="cR")
                        sR = crs.tile([P, SF], F32, tag="sR", name="sR")
                        nc.sync.dma_start(cR[:], d_cosr.ap()[:, js])
                        nc.sync.dma_start(sR[:], d_sinr.ap()[:, js])
                        nc.vector.tensor_mul(t1[:], pss[NHC][:], cR[:])
                        nc.vector.tensor_mul(t2[:], pss[NHC + 1][:], sR[:])
                        nc.vector.tensor_sub(ror[:], t1[:], t2[:])
                        nc.vector.tensor_mul(t1[:], pss[NHC][:], sR[:])
                        nc.vector.tensor_mul(t2[:], pss[NHC + 1][:], cR[:])
                        nc.vector.tensor_add(roi[:], t1[:], t2[:])
                        for h in range(NHC):
                            hs = slice(h * 32, (h + 1) * 32)
                            nc.sync.dma_start(
                                d_qps.ap()[h, 0:32, js], ror[hs, :])
                            nc.sync.dma_start(
                                d_qps.ap()[h, 32:64, js], roi[hs, :])
                        # ---- kv projection for this t block (same x) ----
                        psc = [qac.tile([P, 320], F32, tag="acc",
                                        name="acc", padded_shape=[P, SF])
                               for _ in range(4)]
                        psp = [qac.tile([P, 256], F32, tag="acc",
                                        name="accp",
                                        padded_shape=[P, SF])
                               for _ in range(4)]
                        for d in range(NDC):
                            xs = xh[d // 4]
                            for ti in range(4):
                                xtsl = xs[:, (d % 4) * SF + ti * P:
                                          (d % 4) * SF + (ti + 1) * P]
                                nc.tensor.matmul(
                                    psc[ti][:],
                                    xtsl,
                                    wkva_a[:, d * (KVL + DR):
                                           d * (KVL + DR) + 320],
                                    start=(d == 0), stop=(d == NDC - 1))
                                nc.tensor.matmul(
                                    psp[ti][:],
                                    xtsl,
                                    wkva_a[:, d * (KVL + DR) + 320:
                                           (d + 1) * (KVL + DR)],
                                    start=(d == 0), stop=(d == NDC - 1))
                        for ti in range(4):
                            t = j * 4 + ti
                            sq = sqs.tile([P, KVL], F32, tag="sq", name="sq")
                            ss = nrm.tile([P, 1], F32, tag="ss", name="ss")
                            ss2 = nrm.tile([P, 1], F32, tag="ss2",
                                           name="ss2")
                            nc.scalar.activation(
                                sq[:, 0:320], psc[ti][:],
                                mybir.ActivationFunctionType.Square,
                                accum_out=ss[:])
                            nc.scalar.activation(
                                sq[:, 320:KVL], psp[ti][:, 0:192],
                                mybir.ActivationFunctionType.Square,
                                accum_out=ss2[:])
                            nc.vector.tensor_add(ss[:], ss[:], ss2[:])
                            rt_ = nrm.tile([P, 1], F32, tag="rt", name="rt")
                            nc.scalar.activation(
                                rt_[:], ss[:],
                                mybir.ActivationFunctionType.Sqrt,
                                bias=epsb[:], scale=1.0 / KVL)
                            ri = nrm.tile([P, 1], F32, tag="ri", name="ri")
                            nc.vector.reciprocal(ri[:], rt_[:])
                            nc.scalar.mul(kvc[t][:, 0:320], psc[ti][:],
                                          ri[:])
                            nc.scalar.mul(kvc[t][:, 320:KVL],
                                          psp[ti][:, 0:192], ri[:])
                            # k rope (deinterleave to [r(32) | i(32)])
                            cn = cna[:, t * 32:(t + 1) * 32]
                            sn = sna[:, t * 32:(t + 1) * 32]
                            pe = psp[ti][:, 192:256].rearrange(
                                "p (k two) -> p k two", two=2)
                            xr = pe[:, :, 0:1].rearrange(
                                "p k one -> p (k one)")
                            xi = pe[:, :, 1:2].rearrange(
                                "p k one -> p (k one)")
                            m1 = kct.tile([P, DR // 2], F32, tag="m1",
                                          name="m1")
                            m2 = kct.tile([P, DR // 2], F32, tag="m2",
                                          name="m2")
                            nc.vector.tensor_mul(m1[:], xr, cn)
                            nc.vector.tensor_mul(m2[:], xi, sn)
                            nc.vector.tensor_sub(kpe[t][:, 0:32], m1[:],
                                                 m2[:])
                            nc.vector.tensor_mul(m1[:], xr, sn)
                            nc.vector.tensor_mul(m2[:], xi, cn)
                            nc.vector.tensor_add(kpe[t][:, 32:64], m1[:],
                                                 m2[:])

            # ============ phase 2: transposes + attention ==================
            kvtp = top.enter_context(tc.tile_pool(name="kvT", bufs=NCC))
            kptp = top.enter_context(tc.tile_pool(name="kpT", bufs=1))
            msp = top.enter_context(
                tc.tile_pool(name="msp", bufs=3, space="PSUM"))
            otp = top.enter_context(tc.tile_pool(name="oT", bufs=NHC))
            oTs = [otp.tile([DV, S], RT, tag="oT", name="oT")
                   for _ in range(NHC)]
            kvcT = [kvtp.tile([P, S], RT, tag="kvcT", name="kvcT")
                    for _ in range(NCC)]
            kpeT = kptp.tile([DR, S], RT, tag="kpeT", name="kpeT")
            for t in range(NTT):
                ts_ = slice(t * P, (t + 1) * P)
                for cc in range(NCC):
                    tp = msp.tile([P, SF], F32, tag="msp", name="msp")
                    nc.tensor.transpose(
                        tp[:, 0:P],
                        kvc[t][:, cc * P:(cc + 1) * P].bitcast(F32),
                        ident[:])
                    nc.scalar.copy(kvcT[cc][:, ts_], tp[:, 0:P])
                tp = msp.tile([P, SF], F32, tag="msp", name="msp")
                nc.tensor.transpose(tp[0:DR, 0:P], kpe[t][:], ident[:])
                nc.scalar.copy(kpeT[:, ts_], tp[0:DR, 0:P])

            with ExitStack() as ph2:
                qhp = ph2.enter_context(tc.tile_pool(name="qh", bufs=2))
                qpp = ph2.enter_context(tc.tile_pool(name="qpp", bufs=2))
                wbp = ph2.enter_context(tc.tile_pool(name="wb", bufs=2))
                qap = ph2.enter_context(tc.tile_pool(name="qabs", bufs=8))
                etp = ph2.enter_context(tc.tile_pool(name="et", bufs=6))
                olp = ph2.enter_context(
                    tc.tile_pool(name="olat", bufs=4, space="PSUM"))
                dnp = ph2.enter_context(
                    tc.tile_pool(name="dn", bufs=1, space="PSUM"))
                osp = ph2.enter_context(tc.tile_pool(name="osb", bufs=8))
                dvp = ph2.enter_context(tc.tile_pool(name="dinv", bufs=2))

                for h in range(NHC):
                    qn = qhp.tile([DN, S], RT, tag="qn", name="qn")
                    nc.sync.dma_start(qn[:], d_qns.ap()[h])
                    qp = qpp.tile([DR, S], RT, tag="qp", name="qp")
                    nc.sync.dma_start(qp[:], d_qps.ap()[h])
                    wbk = wbp.tile([DN, KVL], RT, tag="wbk", name="wbk")
                    nc.sync.dma_start(wbk[:], d_wbk.ap()[h])
                    wbv = wbp.tile([P, NCC * DV], RT, tag="wbv", name="wbv")
                    nc.sync.dma_start(
                        wbv[:].rearrange("p (cc dv) -> p cc dv", cc=NCC),
                        d_wbvT.ap()[h].rearrange("(cc p) dv -> p cc dv",
                                                 p=P))
                    for j in range(NST):
                        js = slice(j * SF, (j + 1) * SF)
                        qa = [qap.tile([P, SF], RT, tag="qa", name="qa")
                              for _ in range(NCC)]
                        for cc in range(NCC):
                            ps = msp.tile([P, SF], F32, tag="msp",
                                          name="msp")
                            nc.tensor.matmul(
                                ps[:], wbk[:, cc * P:(cc + 1) * P],
                                qn[:, js], start=True, stop=True)
                            nc.vector.tensor_copy(qa[cc][:], ps[:])
                        ol = [olp.tile([P, SF], F32, tag="olat",
                                       name="olat") for _ in range(NCC)]
                        dn = dnp.tile([1, SF], F32, tag="dn", name="dn")
                        ntt = 4 * j + 4
                        for t in range(ntt):
                            ts_ = slice(t * P, (t + 1) * P)
                            # causal narrowing: diagonal tiles only need
                            # columns s >= t, i.e. local offset 128*(t-4j)
                            off = max(0, min(P * (t - 4 * j), SF - 256))
                            nf = SF - off
                            osl = slice(j * SF + off, (j + 1) * SF)
                            sc = msp.tile([P, SF], F32, tag="msp",
                                          name="msp")
                            for cc in range(NCC):
                                nc.tensor.matmul(
                                    sc[:, 0:nf], kvcT[cc][:, ts_],
                                    qa[cc][:, off:SF],
                                    start=(cc == 0), stop=False)
                            nc.tensor.matmul(
                                sc[:, 0:nf], kpeT[:, ts_], qp[:, osl],
                                start=False, stop=True)
                            e = etp.tile([P, SF], RT, tag="et", name="et")
                            nc.scalar.activation(
                                e[:, 0:nf], sc[:, 0:nf],
                                mybir.ActivationFunctionType.Exp,
                                scale=SCALE)
                            if t >= 4 * j:
                                nc.gpsimd.affine_select(
                                    out=e[:, 0:nf], in_=e[:, 0:nf],
                                    compare_op=mybir.AluOpType.is_ge,
                                    fill=0.0, base=SF * j + off - P * t,
                                    pattern=[[1, nf]],
                                    channel_multiplier=-1)
                            nc.tensor.matmul(
                                dn[:, off:SF], ones_c[:], e[:, 0:nf],
                                start=(t == 0), stop=(t == ntt - 1))
                            for cc in range(NCC):
                                nc.tensor.matmul(
                                    ol[cc][:, off:SF],
                                    kvc[t][:, cc * P:(cc + 1) * P],
                                    e[:, 0:nf], start=(t == 0),
                                    stop=(t == ntt - 1))
                        di = dvp.tile([1, SF], F32, tag="di", name="di")
                        nc.vector.reciprocal(di[:], dn[:])
                        dbp = msp.tile([P, SF], F32, tag="msp", name="msp")
                        nc.tensor.matmul(dbp[:], ones_r[:], di[:],
                                         start=True, stop=True)
                        db = dvp.tile([P, SF], F32, tag="db", name="db")
                        nc.scalar.copy(db[:], dbp[:])
                        osb = [osp.tile([P, SF], RT, tag="osb", name="osb")
                               for _ in range(NCC)]
                        for cc in range(NCC):
                            nc.scalar.copy(osb[cc][:], ol[cc][:])
                        ohps = msp.tile([P, SF], F32, tag="msp", name="msp")
                        for cc in range(NCC):
                            nc.tensor.matmul(
                                ohps[:], wbv[:, cc * DV:(cc + 1) * DV],
                                osb[cc][:],
                                start=(cc == 0), stop=(cc == NCC - 1))
                        nc.vector.tensor_mul(oTs[h][:, js], ohps[:], db[:])

            # ============ phase 3: output projection (partial) =============
            with ExitStack() as ph3:
                wop = ph3.enter_context(tc.tile_pool(name="wo", bufs=NHC))
                otg = ph3.enter_context(tc.tile_pool(name="ost", bufs=3))
                wos = [wop.tile([DV, DIM], RT, tag="wo", name="wo")
                       for _ in range(NHC)]
                for h in range(NHC):
                    nc.sync.dma_start(
                        wos[h][:], d_wo.ap()[h * DV:(h + 1) * DV, :])
                for d in range(NDC):
                    ds_ = slice(d * P, (d + 1) * P)
                    obig = otg.tile([P, S], F32, tag="ost", name="ost")
                    for j in range(NST):
                        js = slice(j * SF, (j + 1) * SF)
                        ps = msp.tile([P, SF], F32, tag="msp", name="msp")
                        for h in range(NHC):
                            nc.tensor.matmul(
                                ps[:], wos[h][:, ds_], oTs[h][:, js],
                                start=(h == 0), stop=(h == NHC - 1))
                        nc.scalar.copy(obig[:, js], ps[:])
                    nc.sync.dma_start(out[ds_, :], obig[:])

    nc.compile()
    return nc


def prep_inputs(x, wq_w, wkv_a_w, wkv_b_w, kv_norm_w, wo_w,
                freqs_cos, freqs_sin):
    """Host-side sharding/layout prep. Returns per-core input maps."""
    x = np.ascontiguousarray(np.asarray(x, np.float32).reshape(S, DIM))
    xT = np.ascontiguousarray(x.T)
    wq = np.asarray(wq_w, np.float32).reshape(DIM, NH, DN + DR)
    wkva = np.ascontiguousarray(np.asarray(wkv_a_w, np.float32))
    wkvb = np.asarray(wkv_b_w, np.float32)
    knw = np.asarray(kv_norm_w, np.float32)
    wo = np.asarray(wo_w, np.float32)
    cos = np.asarray(freqs_cos, np.float32)
    sin = np.asarray(freqs_sin, np.float32)
    cosR = np.ascontiguousarray(np.tile(cos.T, (NHC, 1)))  # [128, S]
    sinR = np.ascontiguousarray(np.tile(sin.T, (NHC, 1)))

    maps = []
    for c in range(NCORES):
        hs = list(range(NHC * c, NHC * (c + 1)))
        wq_n = np.ascontiguousarray(
            wq[:, hs, :DN].reshape(DIM, NHC * DN))
        wq_pr = np.ascontiguousarray(
            wq[:, hs, DN + 0::2].reshape(DIM, NHC * 32))
        wq_pi = np.ascontiguousarray(
            wq[:, hs, DN + 1::2].reshape(DIM, NHC * 32))
        # fold kv_norm weight into the absorbed weights
        wbk = np.stack([wkvb[h * (DN + DV):h * (DN + DV) + DN, :] * knw[None, :]
                        for h in hs])                       # [4, 128, 512]
        wbvT = np.stack(
            [np.ascontiguousarray(
                wkvb[h * (DN + DV) + DN:(h + 1) * (DN + DV), :].T)
             * knw[:, None] for h in hs])                   # [4, 512, 128]
        wo_c = np.ascontiguousarray(
            np.concatenate([wo[h * DV:(h + 1) * DV, :] for h in hs]))
        maps.append({
            "xT": xT, "wq_n": wq_n, "wq_pr": wq_pr, "wq_pi": wq_pi,
            "wkv_a": wkva, "wbk": np.ascontiguousarray(wbk),
            "wbvT": np.ascontiguousarray(wbvT), "wo_c": wo_c,
            "cos_n": cos, "sin_n": sin, "cosR": cosR, "sinR": sinR,
        })
    return maps


def kernel(x, wq_w, wkv_a_w, wkv_b_w, kv_norm_w, wo_w,
           freqs_cos, freqs_sin, start_pos):
    assert int(start_pos) == 0
    maps = prep_inputs(x, wq_w, wkv_a_w, wkv_b_w, kv_norm_w, wo_w,
                       freqs_cos, freqs_sin)
    nc = build_nc()
    res = run_bass_kernel_spmd(nc, maps, list(range(NCORES)))
    acc = np.zeros((DIM, S), np.float64)
    for c in range(NCORES):
        acc += res.results[c]["outT"]
    return np.ascontiguousarray(acc.T).astype(np.float32).reshape(1, S, DIM)

